# revision 54
# baseline (speedup 1.0000x reference)
"""LIF spike-train kernel for Trainium2 (Bass/Tile), data-parallel over 8 cores.

Reference semantics (T=4, tau=0.5, thresh=1.0), per element:
    mem = 0
    for t in range(4):
        mem = mem*0.5 + x[t]
        s[t] = (mem - 1 >= 0)
        mem = mem - s[t]

x: [T*B, C, H, W] = [256, 128, 32, 32] f32, viewed as [4, 64, 128, 1024].
Batch dim (64) is sharded 8-ways; each core streams [4, 8, 128, 1024],
flattened to x,y: [T, 128, F] (F = 8192).

Scheme "psum" (PE/PSUM membrane offload; best 66.7us, quiet ~67, noisy
72-76, vs 75.4us SignFlow baseline):

  Budget (all HW-measured this session): ~8.7us fixed framework preamble
  (2 init barriers + per-engine TENSOR_LOADs + const-table static DMA)
  before the first load byte moves; 16.8MB loads at 410-425GB/s (the
  fabric/HBM ceiling) ending ~50us; then a pipeline-drain tail.  Exec is
  port+latency-bound at ~73-78us; DVE busy is only ~46us (the old scheme
  was DVE-bound at 55-58us busy).

  Per chunk of CW=1024, per step t<3 (membrane v_t = u_t - s_t):
    cmp_t:  s_t = is_ge(u_t, 1.0) -> fp8 {0, 0x38}     DVE 1-src 2x (684ns)
    reset:  B = 0.5*u_t   (Act Copy scale=0.5 -> PSUM, exact)   (1117ns)
            B += -0.5*s_t (PE matmul, bf16 -0.5*I @ fp8 s, exact) (630ns x2)
    integ:  u_{t+1} = stt(B, +, x_{t+1}) -> SBUF       DVE 2-src 1x (1216ns)
    t=3:    sign(u_3 - c) on Act (bias AP), byte {0xB8,0,0x38}
  Rounding matches the reference exactly: 0.5*u and 0.5*s are exact,
  PSUM accumulate gives fl(0.5u - 0.5s) = 0.5*v exactly, and the stt is
  the single rounding fl(0.5v + x) the reference performs.  rel err 0.0.

  KEY HW FACTS (micro-validated in micro_psum.py on this HW):
  - PSUM has a per-element has_written bit that only PE matmuls set; a
    matmul(start=False) OVERWRITES (not accumulates onto) Act-written
    data wherever the bit is clear -- this corrupted all earlier PE
    attempts.  Fix: one dummy matmul(start=True) per PSUM bank in the
    preamble sets the whole bank's bits; nothing ever clears them, so
    Act-write -> matmul(start=False) accumulates correctly forever.
  - fp32 identity matmul is bit-exact (usable for exact PE x-injection,
    4 cyc/row).  fp32r needs pre-rounded inputs (lossy) -- rejected.
  - Dense 3-engine schedules (DVE+Act+PE all busy, t-major rows) clock-
    throttle EVERY engine's ops by exactly 1.2x; the chunk-skewed
    schedule keeps ops at full speed (stt 1216, cmp 684, act 1117).

  Schedule: 8 chunks x 4 steps, software-pipelined with chunk j running
  step t at round j+t => 4 chunks (= 4 PSUM B-tags x 2 banks, bufs=1,
  fresh generation per step) in flight, hiding the ~3.4us/step cross-
  engine chain latency.  The whole input is SBUF-resident (16 x-tiles,
  own tag each) so load triggers carry no compute-gated WARs; loads are
  emitted in DIAGONAL (consumption) order -- key 2p+t, pair 3 advanced
  one slot and split into per-chunk halves -- on a pure-load sync queue
  (stores emitted later can then never delay a load; loads+stores on
  one HWDGE ring drain in order).  Spike tiles are per (t, block-of-4):
  a block store must never share a tile with the other block's pending
  compare writes (tile-granular WAR tracking serializes that, ~1.5us
  spurious DVE stall per store).  Stores are [128,4096] 4KB-row DMAs as
  each block row completes; block 1's t3 goes out per-piece from
  per-piece tiles; the last two chunks run 512-wide sub-chains; t0
  compares for pairs 1-3 are pair-wide (x is contiguous pair-wide, u is
  not).  The last two chunks' FINAL reset (s2) runs inline on the DVE
  into SBUF (nv = s2 - u2; s3 integrate becomes nv*-0.5 + x3, the
  SignFlow form) -- the PSUM form's Act->PE hops sat on the critical
  tail as a measured 2.7us wait on chunk 7's s2 matmuls.  Only terminal
  resets benefit: converting mid-stream resets adds time to the DVE,
  the critical engine.  Host decodes spike := byte == 0x38.

  WITHIN-ROUND EMISSION ORDER (the final 8us): steps of a round are
  emitted in the order their x tiles were LOADED, not chunk-ascending.
  Chunk-ascending order is t-DESCENDING, but the load tie-break ships
  lower-t tiles first, so the round's head instruction needed the
  round's last-arriving tile and ready work stalled behind it in the
  in-order engine queues (~1us per round, ~8us total).  With
  arrival-ordered steps the DVE runs gapless at the load frontier.

  End-state accounting (fast rep, 66.7us): DVE starts ~10.7 (preamble
  8.7 + first split load), runs 46.5us of work essentially gapless to
  +60.8, then sign 0.7 + store/receipt ~2 + drain handshake + final
  barrier => 66.7.  Reps where neighbor-core HBM contention jitters
  the load supply degrade to 72-76 (in-order queues amplify supply
  stutter; op durations stay identical).  DVE is the critical engine
  (~46.5us; Act ~35); PE fp32 x-injection cannot relieve it because PE
  runs at mid-pstate (630ns per 512-row fp8 matmul, ~3x ramped) and
  fp32 is 4x that.  Every exact reformulation of the reset needs fp8
  {0,1} (DVE is_ge only); constant-splitting the step through fp32
  double-rounds (proven), so the compare+integrate DVE floor stands.

Rejected experiments (measured): per-(t,pair) 2KB-row stores ~300GB/s
(descriptor overhead); stores on gpsimd SWDGE (shares the 8 DMA sem
lanes -> load triggers convoy behind compute-gated stores); t-major
block schedule (1.2x clock throttle); B2-evac to SBUF + t3 drain phase
(evac copies queue at the tail of the busy Act queue / extra DVE work
and late t3 loads both lost ~5-9us).
"""

import os
import sys

sys.path.insert(0, "/opt/trn_rl_repo")

import numpy as np

T = 4
B = 64
C = 128
HW = 1024
NCORES = 8
BLOC = B // NCORES  # 8 batch elements per core
F = BLOC * C * HW // 128  # 8192 flat free width per t-block
C_THRESH = float(np.nextafter(np.float32(1.0), np.float32(0.0)))

LAST_EXEC_NS = None
LAST_TRACE = None

_CACHE = {}


def _build_psum():
    """PE/PSUM membrane scheme: DVE cmp+integrate, Act+PE reset."""
    import concourse.bacc as bacc
    import concourse.mybir as mybir
    from concourse import tile

    f32 = mybir.dt.float32
    fp8 = mybir.dt.float8e4
    bf16 = mybir.dt.bfloat16
    A = mybir.AluOpType
    AF = mybir.ActivationFunctionType

    CW = 1024  # compute chunk width (B tile = 2 PSUM banks)
    PW = 2048  # load/store pair width
    NP = F // PW  # 4 pairs
    NBT = int(os.environ.get("LIF_NBT", "4"))  # B tags (chunks in flight)

    xbufs = int(os.environ.get("LIF_XBUFS", "10"))
    ubufs = int(os.environ.get("LIF_UBUFS", "6"))
    t8bufs = int(os.environ.get("LIF_T8BUFS", "2"))
    st_name = os.environ.get("LIF_STORE_ENG", "sync")
    x0split = int(os.environ.get("LIF_X0SPLIT", "2"))  # pair-0 t0 load pieces
    tailsub = int(os.environ.get("LIF_TAILSUB", "512"))  # last-pair t3 grain

    nc = bacc.Bacc("TRN2", target_bir_lowering=False, debug=False, num_devices=NCORES)
    x = nc.dram_tensor("x", [T, 128, F], f32, kind="ExternalInput").ap()
    y = nc.dram_tensor("y", [T, 128, F], fp8, kind="ExternalOutput").ap()
    wm = nc.dram_tensor("w", [128, 128], bf16, kind="ExternalInput").ap()

    with tile.TileContext(nc) as tc:
        with tc.tile_pool(name="p", bufs=xbufs) as pool, tc.psum_pool(
            name="ps", bufs=1
        ) as ppool:
            st = {"sync": nc.sync, "scalar": nc.scalar, "gpsimd": nc.gpsimd}[st_name]

            # wt on the scalar (HWDGE) queue: keeps the sync queue pure-loads
            wt = pool.tile([128, 128], bf16, tag="wt", bufs=1)
            nc.scalar.dma_start(out=wt, in_=wm)
            bias = pool.tile([128, 1], f32, tag="bias", bufs=1)
            warm = pool.tile([128, 1], fp8, tag="warm", bufs=1)
            dummy = pool.tile([128, 512], bf16, tag="dummy", bufs=1)
            nc.vector.memset(bias, -C_THRESH)
            # dummy feeds only the preamble warmup matmuls; memset it on the
            # otherwise-idle gpsimd so the DVE queue reaches cmp0 sooner
            nc.gpsimd.memset(dummy, 0.0)
            # pull the ACT table load into the preamble
            nc.scalar.activation(warm, bias, AF.Sign, bias=bias)

            # PSUM has_written warmup: one start=True matmul per bank.
            # Keep a handle per tag; generations after this reuse the banks.
            for k in range(NBT):
                Bw = ppool.tile([128, CW], f32, tag=f"B{k}", bufs=1, name=f"Bw{k}")
                for h in range(CW // 512):
                    nc.tensor.matmul(
                        Bw[:, h * 512 : (h + 1) * 512],
                        wt,
                        dummy,
                        start=True,
                        stop=True,
                    )

            # ---- software-pipelined emission: chunk j runs step t at
            # round r = j + t, so 4 chunks (= 4 B tags) are in flight and
            # the per-step Act->PE->DVE chain latency is hidden behind the
            # other chunks' DVE work.  The whole input (16 MB/core = 128KB
            # per partition) is SBUF-resident: every x tile gets its own
            # tag so load triggers never carry compute-gated WAR waits --
            # the sync queue is pure loads, streaming at port rate.
            # Stores ride the idle gpsimd SWDGE queue, one round behind.
            NCH = F // CW  # 8 chunks
            xts = {}  # pair -> [xt per t]
            Bs = {}  # chunk -> current B psum tile
            # spike tiles are per (t, block-of-4-chunks): a store of block
            # 0's half must NOT share a tile with block 1's still-pending
            # is_ge writes -- the tile framework serializes that at tile
            # granularity (measured ~1.5us spurious DVE wait per store).
            # Block 1's t3 is further split per-chunk for the drain path.
            HBW = (NCH // 2) * CW  # 4096
            t8s = {}
            for t in range(T):
                for b in range(2):
                    if t == T - 1 and b == 1:
                        continue
                    t8s[(t, b)] = pool.tile(
                        [128, HBW], fp8, tag=f"t8_{t}_{b}", bufs=1,
                        name=f"t8_{t}_{b}",
                    )
            # per-(chunk, piece) drain tiles: a piece's store must never
            # share a tile with a later piece's pending sign write
            t8d = {
                (j, lo): pool.tile(
                    [128, tailsub if j >= NCH - 2 else CW], fp8,
                    tag=f"t8d_{j}_{lo}", bufs=1, name=f"t8d_{j}_{lo}"
                )
                for j in range(NCH // 2, NCH)
                for lo in range(0, CW, tailsub if j >= NCH - 2 else CW)
            }

            # loads in DIAGONAL (consumption) order: tile (p, t) is first
            # read at round 2p+t by chunk 2p, so emit loads sorted by that
            # round -- pair-major order would land every t>=2 tile a few us
            # after its consumer and stall the DVE once per round.
            for p in range(NP):
                xts[p] = [
                    pool.tile(
                        [128, PW], f32, tag=f"x{p}_{t}", bufs=1, name=f"x_{p}_{t}"
                    )
                    for t in range(T)
                ]
            p3adv = int(os.environ.get("LIF_P3ADV", "1"))  # pull pair-3 early

            def load_key(pt):
                p, t = pt
                return (2 * p + t - (p3adv if p == NP - 1 else 0), t)

            for p, t in sorted(
                ((p, t) for p in range(NP) for t in range(T)), key=load_key
            ):
                xt = xts[p][t]
                if p == 0 and t == 0 and x0split > 1:
                    wsub = PW // x0split
                    for k in range(x0split):
                        nc.sync.dma_start(
                            out=xt[:, k * wsub : (k + 1) * wsub],
                            in_=x[0][:, k * wsub : (k + 1) * wsub],
                        )
                elif p == NP - 1 and t > 0:
                    # last pair: per-chunk halves so each drain-phase step
                    # is gated only by its own 1MB, not the whole 2MB
                    for k in range(2):
                        nc.sync.dma_start(
                            out=xt[:, k * CW : (k + 1) * CW],
                            in_=x[t][:, p * PW + k * CW : p * PW + (k + 1) * CW],
                        )
                else:
                    nc.sync.dma_start(out=xt, in_=x[t][:, p * PW : (p + 1) * PW])

            def emit_substep(j, t, lo, hi, u_of):
                """One step of chunk j restricted to columns [lo,hi) of the
                chunk; u_of maps a sub-slice to the u AP."""
                blk = j // (NCH // 2)
                bcol = (j - blk * (NCH // 2)) * CW  # within the block tile
                bsl = slice(bcol + lo, bcol + hi)
                gsl = slice(j * CW + lo, j * CW + hi)  # within y[t]
                u = u_of(lo, hi)
                if t == T - 2 and j >= NCH - 2:
                    # last two chunks: final reset inline on the DVE into
                    # SBUF (nv = max(s2,0) - u2).  The PSUM form's Act->PE
                    # hops (~1us each) sit on the critical tail here --
                    # measured as a 2.7us wait on chunk 7's s2 matmuls.
                    # The compare rides Act's idle tail as a Sign (the
                    # max(s,0) absorbs the -1 bytes; decode is still 0x38),
                    # shifting ~1.7us off the work-bound DVE's end.
                    tt = t8s[(t, blk)]
                    nc.scalar.activation(tt[:, bsl], u, AF.Sign, bias=bias)
                    nc.vector.scalar_tensor_tensor(
                        Bs[(j, t)][:, lo:hi], tt[:, bsl], 0.0, u,
                        A.max, A.subtract,
                    )
                elif t < T - 1:
                    Bn = Bs[(j, t)]
                    tt = t8s[(t, blk)]
                    nc.scalar.activation(Bn[:, lo:hi], u, AF.Copy, 0.0, 0.5)
                    if t > 0 or j // 2 == 0:  # t0 cmp of pairs 1-3 is pair-wide
                        nc.vector.tensor_scalar(tt[:, bsl], u, 1.0, None, A.is_ge)
                    for h in range(lo // 512, hi // 512):
                        nc.tensor.matmul(
                            Bn[:, h * 512 : (h + 1) * 512],
                            wt,
                            tt[:, bcol + h * 512 : bcol + (h + 1) * 512],
                            start=False,
                            stop=True,
                            skip_group_check=True,
                        )
                elif blk == 0:
                    nc.scalar.activation(
                        t8s[(3, 0)][:, bsl], u, AF.Sign, bias=bias
                    )
                else:
                    # block 1 t3: per-piece tile, stored as soon as signed
                    td = t8d[(j, lo)]
                    nc.scalar.activation(td, u, AF.Sign, bias=bias)
                    nc.sync.dma_start(out=y[3][:, gsl], in_=td)

            def emit_step(j, t):
                p, ci = j // 2, j % 2
                xsl = slice(ci * CW, (ci + 1) * CW)
                # sub-chain granularity: last two chunks run 512-wide so
                # the drain chain latency after the final loads is halved
                grain = tailsub if j >= NCH - 2 else CW
                if t == 0:
                    u_of = lambda lo, hi: xts[p][0][:, ci * CW + lo : ci * CW + hi]
                    # pair-wide t0 compare: x is contiguous across the pair,
                    # so one [128,2048] is_ge replaces two (saves the DVE
                    # per-op init cost); pair 0 keeps per-chunk (split loads)
                    if p > 0 and ci == 0:
                        blk = j // (NCH // 2)
                        bcol = (j - blk * (NCH // 2)) * CW
                        nc.vector.tensor_scalar(
                            t8s[(0, blk)][:, bcol : bcol + PW],
                            xts[p][0],
                            1.0,
                            None,
                            A.is_ge,
                        )
                else:
                    un = pool.tile(
                        [128, CW], f32, tag="u", bufs=ubufs, name=f"u_{j}_{t}"
                    )
                    u_of = lambda lo, hi: un[:, lo:hi]
                if t == T - 2 and j >= NCH - 2:
                    # SBUF nv tile for the inline DVE reset (see emit_substep)
                    Bs[(j, t)] = pool.tile(
                        [128, CW], f32, tag=f"nv{j}", bufs=1, name=f"nv{j}"
                    )
                elif t < T - 1:
                    Bs[(j, t)] = ppool.tile(
                        [128, CW], f32, tag=f"B{j % NBT}", bufs=1, name=f"B{j}_{t}"
                    )
                # chunks 6-7's s3 integrate reads the SBUF nv (= s2 - u2)
                # from the inline DVE reset: u3 = nv * -0.5 + x3
                nvform = t == T - 1 and j >= NCH - 2
                for lo in range(0, CW, grain):
                    hi = lo + grain
                    if t > 0:
                        nc.vector.scalar_tensor_tensor(
                            un[:, lo:hi],
                            Bs[(j, t - 1)][:, lo:hi],
                            -0.5 if nvform else 0.0,
                            xts[p][t][:, ci * CW + lo : ci * CW + hi],
                            A.mult if nvform else A.add,
                            A.add,
                        )
                    emit_substep(j, t, lo, hi, u_of)

            # skewed software pipeline: chunk j runs step t at round j+t,
            # 4 chunks (= 4 B tags) in flight; block stores at the rounds
            # where each half of t8[t] completes
            for r in range(NCH + T - 1):
                # within a round, emit steps in the order their x tiles were
                # LOADED (the same load_key): chunk-ascending order puts the
                # round's last-arriving tile at the queue head and ready
                # work stalls behind it (in-order engine queues)
                steps = [(j, r - j) for j in range(NCH) if 0 <= r - j < T]
                steps.sort(key=lambda jt: load_key((jt[0] // 2, jt[1])))
                for j, t in steps:
                    emit_step(j, t)
                # t8[(t, blk)] complete once its last chunk passes t:
                # block 0 (chunks 0-3) at round 3+t, block 1 at round 7+t.
                # Block 1's t3 goes out per-chunk inside emit_substep.
                for blk, lastj in ((0, NCH // 2 - 1), (1, NCH - 1)):
                    t = r - lastj
                    if 0 <= t < T and not (blk == 1 and t == T - 1):
                        bs = slice(blk * HBW, (blk + 1) * HBW)
                        st.dma_start(out=y[t][:, bs], in_=t8s[(t, blk)])

    nc.compile()
    return nc


def _build_sign():
    """Fallback: previous SignFlow scheme (~75.4us). See git history of the
    docstring for details; kept for A/B via LIF_SCHEME=sign."""
    import concourse.bacc as bacc
    import concourse.mybir as mybir
    from concourse import tile

    f32 = mybir.dt.float32
    i8 = mybir.dt.int8
    A = mybir.AluOpType
    AF = mybir.ActivationFunctionType

    W = min(int(os.environ.get("LIF_W", "2048")), F)
    CW = min(int(os.environ.get("LIF_CW", str(W))), W)
    NCH = F // W
    SUB = W // CW
    NCC = F // CW
    assert F % W == 0 and W % CW == 0

    nc = bacc.Bacc("TRN2", target_bir_lowering=False, debug=False, num_devices=NCORES)
    x = nc.dram_tensor("x", [T, 128, F], f32, kind="ExternalInput").ap()
    y = nc.dram_tensor("y", [T, 128, F], i8, kind="ExternalOutput").ap()

    xbufs = int(os.environ.get("LIF_XBUFS", "6"))
    ubufs = int(os.environ.get("LIF_UBUFS", "6"))
    tbufs = int(os.environ.get("LIF_TBUFS", "2"))

    with tile.TileContext(nc) as tc:
        with tc.tile_pool(name="p", bufs=xbufs) as pool:
            bias = pool.tile([128, 1], f32, tag="bias", bufs=1)
            warm = pool.tile([128, 1], i8, tag="warm", bufs=1)
            nc.vector.memset(bias, -C_THRESH)
            nc.scalar.activation(warm, bias, AF.Sign, bias=bias)

            W0 = min(int(os.environ.get("LIF_W0", str(CW))), CW)
            nvs = {}
            store_pending = None
            for t in range(T):
                xs = {}
                if t == 0 and W0 < CW:
                    t8 = pool.tile([128, F], i8, tag="t8", bufs=tbufs)
                    for j in range(NCC):
                        nvs[j] = pool.tile(
                            [128, CW], f32, tag=f"nv{j}", bufs=2, name=f"nv0_{j}"
                        )
                    for k in range(F // W0):
                        xt = pool.tile([128, W0], f32, tag="x0", bufs=6)
                        nc.sync.dma_start(out=xt, in_=x[0][:, k * W0 : (k + 1) * W0])
                        sl = slice(k * W0, (k + 1) * W0)
                        nc.scalar.activation(t8[:, sl], xt, AF.Sign, bias=bias)
                        j = (k * W0) // CW
                        nsub = slice(k * W0 - j * CW, (k + 1) * W0 - j * CW)
                        nc.vector.scalar_tensor_tensor(
                            nvs[j][:, nsub], t8[:, sl], 0.0, xt, A.max, A.subtract
                        )
                    nc.sync.dma_start(out=y[0], in_=t8)
                    continue
                for i in range(NCH):
                    xt = pool.tile([128, W], f32, tag="x")
                    nc.sync.dma_start(out=xt, in_=x[t][:, i * W : (i + 1) * W])
                    xs[i] = xt

                if store_pending is not None:
                    pt, pt8 = store_pending
                    nc.sync.dma_start(out=y[pt], in_=pt8)
                    store_pending = None

                t8 = pool.tile([128, F], i8, tag="t8", bufs=tbufs)
                pending = None

                def emit_reset(j, u):
                    sl = slice(j * CW, (j + 1) * CW)
                    nv = pool.tile([128, CW], f32, tag=f"nv{j}", bufs=2)
                    nc.vector.scalar_tensor_tensor(
                        nv, t8[:, sl], 0.0, u, A.max, A.subtract
                    )
                    nvs[j] = nv

                for j in range(NCC):
                    sl = slice(j * CW, (j + 1) * CW)
                    xsl = xs[j // SUB][:, (j % SUB) * CW : (j % SUB + 1) * CW]
                    if t == T - 1 and j == NCC - 1:
                        u = pool.tile([128, CW], f32, tag="u", bufs=ubufs)
                        nq = 4
                        q = CW // nq
                        for k in range(nq):
                            usub = slice(k * q, (k + 1) * q)
                            ysub = slice(j * CW + k * q, j * CW + (k + 1) * q)
                            nc.vector.scalar_tensor_tensor(
                                u[:, usub], nvs[j][:, usub], -0.5,
                                xsl[:, usub], A.mult, A.add,
                            )
                            nc.scalar.activation(
                                t8[:, ysub], u[:, usub], AF.Sign, bias=bias
                            )
                            nc.sync.dma_start(out=y[t][:, ysub], in_=t8[:, ysub])
                        continue
                    if t == 0:
                        u = xsl
                    else:
                        u = pool.tile([128, CW], f32, tag="u", bufs=ubufs)
                        nc.vector.scalar_tensor_tensor(
                            u, nvs[j], -0.5, xsl, A.mult, A.add
                        )
                    if t == 0 and j < 2:
                        nc.vector.tensor_scalar(t8[:, sl], u, 1.0, None, A.is_ge)
                    else:
                        nc.scalar.activation(t8[:, sl], u, AF.Sign, bias=bias)
                    if t < T - 1:
                        if pending is not None:
                            emit_reset(*pending)
                        pending = (j, u)
                    else:
                        nc.sync.dma_start(out=y[t][:, sl], in_=t8[:, sl])
                if pending is not None:
                    emit_reset(*pending)

                if t < T - 1:
                    store_pending = (t, t8)
            if store_pending is not None:
                pt, pt8 = store_pending
                nc.sync.dma_start(out=y[pt], in_=pt8)

    nc.compile()
    return nc


def _get_nc():
    if "nc" not in _CACHE:
        scheme = os.environ.get("LIF_SCHEME", "psum")
        _CACHE["scheme"] = scheme
        _CACHE["nc"] = _build_sign() if scheme == "sign" else _build_psum()
    return _CACHE["nc"]


def kernel(x: np.ndarray) -> np.ndarray:
    global LAST_EXEC_NS, LAST_TRACE
    from concourse.bass_utils import run_bass_kernel_spmd

    x = np.ascontiguousarray(np.asarray(x), dtype=np.float32)
    assert x.shape == (T * B, C, 32, 32), x.shape
    xv = x.reshape(T, B, C, HW)

    nc = _get_nc()
    scheme = _CACHE.get("scheme", "psum")

    wI = None
    if scheme != "sign":
        import ml_dtypes

        wI = (np.eye(128, dtype=np.float32) * -0.5).astype(ml_dtypes.bfloat16)

    in_maps = []
    for m in range(NCORES):
        shard = np.ascontiguousarray(xv[:, m * BLOC : (m + 1) * BLOC]).reshape(
            T, 128, F
        )
        im = {"x": shard}
        if wI is not None:
            im["w"] = wI
        in_maps.append(im)

    trace = os.environ.get("LIF_TRACE") == "1"
    res = run_bass_kernel_spmd(nc, in_maps, core_ids=list(range(NCORES)), trace=trace)
    LAST_EXEC_NS = res.exec_time_ns
    if res.instructions_and_trace is not None:
        LAST_TRACE = res.instructions_and_trace[1]

    out = np.empty((T, B, C, HW), dtype=np.float32)
    for m in range(NCORES):
        raw = np.asarray(res.results[m]["y"])
        if scheme == "sign":
            sp = raw.view(np.int8) == 1
        else:
            # fp8e4 bytes: 1.0 = 0x38 (spike); 0x00 / 0xB8 (-1.0) = no spike
            sp = raw.view(np.uint8) == 0x38
        out[:, m * BLOC : (m + 1) * BLOC] = sp.astype(np.float32).reshape(
            T, BLOC, C, HW
        )
    return out.reshape(T * B, C, 32, 32)


# revision 55
# speedup vs baseline: 1.0795x; 1.0795x over previous
"""LIF spike-train kernel for Trainium2 (Bass/Tile), data-parallel over 8 cores.

Reference semantics (T=4, tau=0.5, thresh=1.0), per element:
    mem = 0
    for t in range(4):
        mem = mem*0.5 + x[t]
        s[t] = (mem - 1 >= 0)
        mem = mem - s[t]

x: [T*B, C, H, W] = [256, 128, 32, 32] f32, viewed as [4, 64, 128, 1024].
Batch dim (64) is sharded 8-ways; each core streams [4, 8, 128, 1024],
flattened to x,y: [T, 128, F] (F = 8192).

Scheme "psum" (PE/PSUM membrane offload; best 66.7us, quiet ~67, noisy
72-76, vs 75.4us SignFlow baseline):

  Budget (all HW-measured this session): ~8.7us fixed framework preamble
  (2 init barriers + per-engine TENSOR_LOADs + const-table static DMA)
  before the first load byte moves; 16.8MB loads at 410-425GB/s (the
  fabric/HBM ceiling) ending ~50us; then a pipeline-drain tail.  Exec is
  port+latency-bound at ~73-78us; DVE busy is only ~46us (the old scheme
  was DVE-bound at 55-58us busy).

  Per chunk of CW=1024, per step t<3 (membrane v_t = u_t - s_t):
    cmp_t:  s_t = is_ge(u_t, 1.0) -> fp8 {0, 0x38}     DVE 1-src 2x (684ns)
    reset:  B = 0.5*u_t   (Act Copy scale=0.5 -> PSUM, exact)   (1117ns)
            B += -0.5*s_t (PE matmul, bf16 -0.5*I @ fp8 s, exact) (630ns x2)
    integ:  u_{t+1} = stt(B, +, x_{t+1}) -> SBUF       DVE 2-src 1x (1216ns)
    t=3:    sign(u_3 - c) on Act (bias AP), byte {0xB8,0,0x38}
  Rounding matches the reference exactly: 0.5*u and 0.5*s are exact,
  PSUM accumulate gives fl(0.5u - 0.5s) = 0.5*v exactly, and the stt is
  the single rounding fl(0.5v + x) the reference performs.  rel err 0.0.

  KEY HW FACTS (micro-validated in micro_psum.py on this HW):
  - PSUM has a per-element has_written bit that only PE matmuls set; a
    matmul(start=False) OVERWRITES (not accumulates onto) Act-written
    data wherever the bit is clear -- this corrupted all earlier PE
    attempts.  Fix: one dummy matmul(start=True) per PSUM bank in the
    preamble sets the whole bank's bits; nothing ever clears them, so
    Act-write -> matmul(start=False) accumulates correctly forever.
  - fp32 identity matmul is bit-exact (usable for exact PE x-injection,
    4 cyc/row).  fp32r needs pre-rounded inputs (lossy) -- rejected.
  - Dense 3-engine schedules (DVE+Act+PE all busy, t-major rows) clock-
    throttle EVERY engine's ops by exactly 1.2x; the chunk-skewed
    schedule keeps ops at full speed (stt 1216, cmp 684, act 1117).

  Schedule: 8 chunks x 4 steps, software-pipelined with chunk j running
  step t at round j+t => 4 chunks (= 4 PSUM B-tags x 2 banks, bufs=1,
  fresh generation per step) in flight, hiding the ~3.4us/step cross-
  engine chain latency.  The whole input is SBUF-resident (16 x-tiles,
  own tag each) so load triggers carry no compute-gated WARs; loads are
  emitted in DIAGONAL (consumption) order -- key 2p+t, pair 3 advanced
  one slot and split into per-chunk halves -- on a pure-load sync queue
  (stores emitted later can then never delay a load; loads+stores on
  one HWDGE ring drain in order).  Spike tiles are per (t, block-of-4):
  a block store must never share a tile with the other block's pending
  compare writes (tile-granular WAR tracking serializes that, ~1.5us
  spurious DVE stall per store).  Stores are [128,4096] 4KB-row DMAs as
  each block row completes; block 1's t3 goes out per-piece from
  per-piece tiles; the last two chunks run 512-wide sub-chains; t0
  compares for pairs 1-3 are pair-wide (x is contiguous pair-wide, u is
  not).  The last two chunks' FINAL reset (s2) runs inline on the DVE
  into SBUF (nv = s2 - u2; s3 integrate becomes nv*-0.5 + x3, the
  SignFlow form) -- the PSUM form's Act->PE hops sat on the critical
  tail as a measured 2.7us wait on chunk 7's s2 matmuls.  Only terminal
  resets benefit: converting mid-stream resets adds time to the DVE,
  the critical engine.  Host decodes spike := byte == 0x38.

  WITHIN-ROUND EMISSION ORDER (the final 8us): steps of a round are
  emitted in the order their x tiles were LOADED, not chunk-ascending.
  Chunk-ascending order is t-DESCENDING, but the load tie-break ships
  lower-t tiles first, so the round's head instruction needed the
  round's last-arriving tile and ready work stalled behind it in the
  in-order engine queues (~1us per round, ~8us total).  With
  arrival-ordered steps the DVE runs gapless at the load frontier.

  End-state accounting (fast rep, 66.7us): DVE starts ~10.7 (preamble
  8.7 + first split load), runs 46.5us of work essentially gapless to
  +60.8, then sign 0.7 + store/receipt ~2 + drain handshake + final
  barrier => 66.7.  Reps where neighbor-core HBM contention jitters
  the load supply degrade to 72-76 (in-order queues amplify supply
  stutter; op durations stay identical).  DVE is the critical engine
  (~46.5us; Act ~35); PE fp32 x-injection cannot relieve it because PE
  runs at mid-pstate (630ns per 512-row fp8 matmul, ~3x ramped) and
  fp32 is 4x that.  Every exact reformulation of the reset needs fp8
  {0,1} (DVE is_ge only); constant-splitting the step through fp32
  double-rounds (proven), so the compare+integrate DVE floor stands.

Rejected experiments (measured): per-(t,pair) 2KB-row stores ~300GB/s
(descriptor overhead); stores on gpsimd SWDGE (shares the 8 DMA sem
lanes -> load triggers convoy behind compute-gated stores); t-major
block schedule (1.2x clock throttle); B2-evac to SBUF + t3 drain phase
(evac copies queue at the tail of the busy Act queue / extra DVE work
and late t3 loads both lost ~5-9us).
"""

import os
import sys

sys.path.insert(0, "/opt/trn_rl_repo")

import numpy as np

T = 4
B = 64
C = 128
HW = 1024
NCORES = 8
BLOC = B // NCORES  # 8 batch elements per core
F = BLOC * C * HW // 128  # 8192 flat free width per t-block
C_THRESH = float(np.nextafter(np.float32(1.0), np.float32(0.0)))

LAST_EXEC_NS = None
LAST_TRACE = None

_CACHE = {}


def _build_psum():
    """PE/PSUM membrane scheme: DVE cmp+integrate, Act+PE reset."""
    import concourse.bacc as bacc
    import concourse.mybir as mybir
    from concourse import tile

    f32 = mybir.dt.float32
    fp8 = mybir.dt.float8e4
    bf16 = mybir.dt.bfloat16
    A = mybir.AluOpType
    AF = mybir.ActivationFunctionType

    CW = 1024  # compute chunk width (B tile = 2 PSUM banks)
    NVCHUNKS = int(os.environ.get("LIF_NVCHUNKS", "2"))  # inline-reset tail chunks
    PW = 2048  # load/store pair width
    NP = F // PW  # 4 pairs
    NBT = int(os.environ.get("LIF_NBT", "4"))  # B tags (chunks in flight)

    xbufs = int(os.environ.get("LIF_XBUFS", "10"))
    ubufs = int(os.environ.get("LIF_UBUFS", "6"))
    t8bufs = int(os.environ.get("LIF_T8BUFS", "2"))
    st_name = os.environ.get("LIF_STORE_ENG", "sync")
    x0split = int(os.environ.get("LIF_X0SPLIT", "2"))  # pair-0 t0 load pieces
    tailsub = int(os.environ.get("LIF_TAILSUB", "512"))  # last-pair t3 grain

    nc = bacc.Bacc("TRN2", target_bir_lowering=False, debug=False, num_devices=NCORES)
    x = nc.dram_tensor("x", [T, 128, F], f32, kind="ExternalInput").ap()
    y = nc.dram_tensor("y", [T, 128, F], fp8, kind="ExternalOutput").ap()
    wm = nc.dram_tensor("w", [128, 128], bf16, kind="ExternalInput").ap()

    with tile.TileContext(nc) as tc:
        with tc.tile_pool(name="p", bufs=xbufs) as pool, tc.psum_pool(
            name="ps", bufs=1
        ) as ppool:
            st = {"sync": nc.sync, "scalar": nc.scalar, "gpsimd": nc.gpsimd}[st_name]

            # wt on the scalar (HWDGE) queue: keeps the sync queue pure-loads
            wt = pool.tile([128, 128], bf16, tag="wt", bufs=1)
            nc.scalar.dma_start(out=wt, in_=wm)
            bias = pool.tile([128, 1], f32, tag="bias", bufs=1)
            warm = pool.tile([128, 1], fp8, tag="warm", bufs=1)
            dummy = pool.tile([128, 512], bf16, tag="dummy", bufs=1)
            nc.vector.memset(bias, -C_THRESH)
            # dummy feeds only the preamble warmup matmuls; memset it on the
            # otherwise-idle gpsimd so the DVE queue reaches cmp0 sooner
            nc.gpsimd.memset(dummy, 0.0)
            # pull the ACT table load into the preamble
            nc.scalar.activation(warm, bias, AF.Sign, bias=bias)

            # PSUM has_written warmup: one start=True matmul per bank.
            # Keep a handle per tag; generations after this reuse the banks.
            for k in range(NBT):
                Bw = ppool.tile([128, CW], f32, tag=f"B{k}", bufs=1, name=f"Bw{k}")
                for h in range(CW // 512):
                    nc.tensor.matmul(
                        Bw[:, h * 512 : (h + 1) * 512],
                        wt,
                        dummy,
                        start=True,
                        stop=True,
                    )

            # ---- software-pipelined emission: chunk j runs step t at
            # round r = j + t, so 4 chunks (= 4 B tags) are in flight and
            # the per-step Act->PE->DVE chain latency is hidden behind the
            # other chunks' DVE work.  The whole input (16 MB/core = 128KB
            # per partition) is SBUF-resident: every x tile gets its own
            # tag so load triggers never carry compute-gated WAR waits --
            # the sync queue is pure loads, streaming at port rate.
            # Stores ride the idle gpsimd SWDGE queue, one round behind.
            NCH = F // CW  # 8 chunks
            xts = {}  # pair -> [xt per t]
            Bs = {}  # chunk -> current B psum tile
            # spike tiles are per (t, block-of-4-chunks): a store of block
            # 0's half must NOT share a tile with block 1's still-pending
            # is_ge writes -- the tile framework serializes that at tile
            # granularity (measured ~1.5us spurious DVE wait per store).
            # Block 1's t3 is further split per-chunk for the drain path.
            HBW = (NCH // 2) * CW  # 4096
            t8s = {}
            for t in range(T):
                for b in range(2):
                    if t == T - 1 and b == 1:
                        continue
                    t8s[(t, b)] = pool.tile(
                        [128, HBW], fp8, tag=f"t8_{t}_{b}", bufs=1,
                        name=f"t8_{t}_{b}",
                    )
            # per-(chunk, piece) drain tiles: a piece's store must never
            # share a tile with a later piece's pending sign write
            t8d = {
                (j, lo): pool.tile(
                    [128, tailsub if j >= NCH - NVCHUNKS else CW], fp8,
                    tag=f"t8d_{j}_{lo}", bufs=1, name=f"t8d_{j}_{lo}"
                )
                for j in range(NCH // 2, NCH)
                for lo in range(0, CW, tailsub if j >= NCH - NVCHUNKS else CW)
            }

            # loads in DIAGONAL (consumption) order: tile (p, t) is first
            # read at round 2p+t by chunk 2p, so emit loads sorted by that
            # round -- pair-major order would land every t>=2 tile a few us
            # after its consumer and stall the DVE once per round.
            for p in range(NP):
                xts[p] = [
                    pool.tile(
                        [128, PW], f32, tag=f"x{p}_{t}", bufs=1, name=f"x_{p}_{t}"
                    )
                    for t in range(T)
                ]
            p3adv = int(os.environ.get("LIF_P3ADV", "1"))  # pull pair-3 early

            def load_key(pt):
                p, t = pt
                return (2 * p + t - (p3adv if p == NP - 1 else 0), t)

            for p, t in sorted(
                ((p, t) for p in range(NP) for t in range(T)), key=load_key
            ):
                xt = xts[p][t]
                if p == 0 and t == 0 and x0split > 1:
                    wsub = PW // x0split
                    for k in range(x0split):
                        nc.sync.dma_start(
                            out=xt[:, k * wsub : (k + 1) * wsub],
                            in_=x[0][:, k * wsub : (k + 1) * wsub],
                        )
                elif p == NP - 1 and t > 0:
                    # last pair: per-chunk halves so each drain-phase step
                    # is gated only by its own 1MB, not the whole 2MB
                    for k in range(2):
                        nc.sync.dma_start(
                            out=xt[:, k * CW : (k + 1) * CW],
                            in_=x[t][:, p * PW + k * CW : p * PW + (k + 1) * CW],
                        )
                else:
                    nc.sync.dma_start(out=xt, in_=x[t][:, p * PW : (p + 1) * PW])

            def emit_substep(j, t, lo, hi, u_of):
                """One step of chunk j restricted to columns [lo,hi) of the
                chunk; u_of maps a sub-slice to the u AP."""
                blk = j // (NCH // 2)
                bcol = (j - blk * (NCH // 2)) * CW  # within the block tile
                bsl = slice(bcol + lo, bcol + hi)
                gsl = slice(j * CW + lo, j * CW + hi)  # within y[t]
                u = u_of(lo, hi)
                if t == T - 2 and j >= NCH - NVCHUNKS:
                    # last two chunks: final reset inline on the DVE into
                    # SBUF (nv = max(s2,0) - u2).  The PSUM form's Act->PE
                    # hops (~1us each) sit on the critical tail here --
                    # measured as a 2.7us wait on chunk 7's s2 matmuls.
                    # The compare rides Act's idle tail as a Sign (the
                    # max(s,0) absorbs the -1 bytes; decode is still 0x38),
                    # shifting ~1.7us off the work-bound DVE's end.
                    tt = t8s[(t, blk)]
                    nc.scalar.activation(tt[:, bsl], u, AF.Sign, bias=bias)
                    nc.vector.scalar_tensor_tensor(
                        Bs[(j, t)][:, lo:hi], tt[:, bsl], 0.0, u,
                        A.max, A.subtract,
                    )
                elif t < T - 1:
                    Bn = Bs[(j, t)]
                    tt = t8s[(t, blk)]
                    nc.scalar.activation(Bn[:, lo:hi], u, AF.Copy, 0.0, 0.5)
                    if t > 0 or j // 2 == 0:  # t0 cmp of pairs 1-3 is pair-wide
                        nc.vector.tensor_scalar(tt[:, bsl], u, 1.0, None, A.is_ge)
                    for h in range(lo // 512, hi // 512):
                        nc.tensor.matmul(
                            Bn[:, h * 512 : (h + 1) * 512],
                            wt,
                            tt[:, bcol + h * 512 : bcol + (h + 1) * 512],
                            start=False,
                            stop=True,
                            skip_group_check=True,
                        )
                elif blk == 0:
                    nc.scalar.activation(
                        t8s[(3, 0)][:, bsl], u, AF.Sign, bias=bias
                    )
                else:
                    # block 1 t3: per-piece tile, stored as soon as signed
                    td = t8d[(j, lo)]
                    nc.scalar.activation(td, u, AF.Sign, bias=bias)
                    nc.sync.dma_start(out=y[3][:, gsl], in_=td)

            def emit_step(j, t):
                p, ci = j // 2, j % 2
                xsl = slice(ci * CW, (ci + 1) * CW)
                # sub-chain granularity: last two chunks run 512-wide so
                # the drain chain latency after the final loads is halved
                grain = tailsub if j >= NCH - NVCHUNKS else CW
                if t == 0:
                    u_of = lambda lo, hi: xts[p][0][:, ci * CW + lo : ci * CW + hi]
                    # pair-wide t0 compare: x is contiguous across the pair,
                    # so one [128,2048] is_ge replaces two (saves the DVE
                    # per-op init cost); pair 0 keeps per-chunk (split loads)
                    if p > 0 and ci == 0:
                        blk = j // (NCH // 2)
                        bcol = (j - blk * (NCH // 2)) * CW
                        nc.vector.tensor_scalar(
                            t8s[(0, blk)][:, bcol : bcol + PW],
                            xts[p][0],
                            1.0,
                            None,
                            A.is_ge,
                        )
                else:
                    un = pool.tile(
                        [128, CW], f32, tag="u", bufs=ubufs, name=f"u_{j}_{t}"
                    )
                    u_of = lambda lo, hi: un[:, lo:hi]
                if t == T - 2 and j >= NCH - NVCHUNKS:
                    # SBUF nv tile for the inline DVE reset (see emit_substep)
                    Bs[(j, t)] = pool.tile(
                        [128, CW], f32, tag=f"nv{j}", bufs=1, name=f"nv{j}"
                    )
                elif t < T - 1:
                    Bs[(j, t)] = ppool.tile(
                        [128, CW], f32, tag=f"B{j % NBT}", bufs=1, name=f"B{j}_{t}"
                    )
                # chunks 6-7's s3 integrate reads the SBUF nv (= s2 - u2)
                # from the inline DVE reset: u3 = nv * -0.5 + x3
                nvform = t == T - 1 and j >= NCH - NVCHUNKS
                for lo in range(0, CW, grain):
                    hi = lo + grain
                    if t > 0:
                        nc.vector.scalar_tensor_tensor(
                            un[:, lo:hi],
                            Bs[(j, t - 1)][:, lo:hi],
                            -0.5 if nvform else 0.0,
                            xts[p][t][:, ci * CW + lo : ci * CW + hi],
                            A.mult if nvform else A.add,
                            A.add,
                        )
                    emit_substep(j, t, lo, hi, u_of)

            # skewed software pipeline: chunk j runs step t at round j+t,
            # 4 chunks (= 4 B tags) in flight; block stores at the rounds
            # where each half of t8[t] completes
            for r in range(NCH + T - 1):
                # within a round, emit steps in the order their x tiles were
                # LOADED (the same load_key): chunk-ascending order puts the
                # round's last-arriving tile at the queue head and ready
                # work stalls behind it (in-order engine queues)
                steps = [(j, r - j) for j in range(NCH) if 0 <= r - j < T]
                steps.sort(key=lambda jt: load_key((jt[0] // 2, jt[1])))
                for j, t in steps:
                    emit_step(j, t)
                # t8[(t, blk)] complete once its last chunk passes t:
                # block 0 (chunks 0-3) at round 3+t, block 1 at round 7+t.
                # Block 1's t3 goes out per-chunk inside emit_substep.
                for blk, lastj in ((0, NCH // 2 - 1), (1, NCH - 1)):
                    t = r - lastj
                    if 0 <= t < T and not (blk == 1 and t == T - 1):
                        bs = slice(blk * HBW, (blk + 1) * HBW)
                        st.dma_start(out=y[t][:, bs], in_=t8s[(t, blk)])

    nc.compile()
    return nc


def _build_sign():
    """Fallback: previous SignFlow scheme (~75.4us). See git history of the
    docstring for details; kept for A/B via LIF_SCHEME=sign."""
    import concourse.bacc as bacc
    import concourse.mybir as mybir
    from concourse import tile

    f32 = mybir.dt.float32
    i8 = mybir.dt.int8
    A = mybir.AluOpType
    AF = mybir.ActivationFunctionType

    W = min(int(os.environ.get("LIF_W", "2048")), F)
    CW = min(int(os.environ.get("LIF_CW", str(W))), W)
    NCH = F // W
    SUB = W // CW
    NCC = F // CW
    assert F % W == 0 and W % CW == 0

    nc = bacc.Bacc("TRN2", target_bir_lowering=False, debug=False, num_devices=NCORES)
    x = nc.dram_tensor("x", [T, 128, F], f32, kind="ExternalInput").ap()
    y = nc.dram_tensor("y", [T, 128, F], i8, kind="ExternalOutput").ap()

    xbufs = int(os.environ.get("LIF_XBUFS", "6"))
    ubufs = int(os.environ.get("LIF_UBUFS", "6"))
    tbufs = int(os.environ.get("LIF_TBUFS", "2"))

    with tile.TileContext(nc) as tc:
        with tc.tile_pool(name="p", bufs=xbufs) as pool:
            bias = pool.tile([128, 1], f32, tag="bias", bufs=1)
            warm = pool.tile([128, 1], i8, tag="warm", bufs=1)
            nc.vector.memset(bias, -C_THRESH)
            nc.scalar.activation(warm, bias, AF.Sign, bias=bias)

            W0 = min(int(os.environ.get("LIF_W0", str(CW))), CW)
            nvs = {}
            store_pending = None
            for t in range(T):
                xs = {}
                if t == 0 and W0 < CW:
                    t8 = pool.tile([128, F], i8, tag="t8", bufs=tbufs)
                    for j in range(NCC):
                        nvs[j] = pool.tile(
                            [128, CW], f32, tag=f"nv{j}", bufs=2, name=f"nv0_{j}"
                        )
                    for k in range(F // W0):
                        xt = pool.tile([128, W0], f32, tag="x0", bufs=6)
                        nc.sync.dma_start(out=xt, in_=x[0][:, k * W0 : (k + 1) * W0])
                        sl = slice(k * W0, (k + 1) * W0)
                        nc.scalar.activation(t8[:, sl], xt, AF.Sign, bias=bias)
                        j = (k * W0) // CW
                        nsub = slice(k * W0 - j * CW, (k + 1) * W0 - j * CW)
                        nc.vector.scalar_tensor_tensor(
                            nvs[j][:, nsub], t8[:, sl], 0.0, xt, A.max, A.subtract
                        )
                    nc.sync.dma_start(out=y[0], in_=t8)
                    continue
                for i in range(NCH):
                    xt = pool.tile([128, W], f32, tag="x")
                    nc.sync.dma_start(out=xt, in_=x[t][:, i * W : (i + 1) * W])
                    xs[i] = xt

                if store_pending is not None:
                    pt, pt8 = store_pending
                    nc.sync.dma_start(out=y[pt], in_=pt8)
                    store_pending = None

                t8 = pool.tile([128, F], i8, tag="t8", bufs=tbufs)
                pending = None

                def emit_reset(j, u):
                    sl = slice(j * CW, (j + 1) * CW)
                    nv = pool.tile([128, CW], f32, tag=f"nv{j}", bufs=2)
                    nc.vector.scalar_tensor_tensor(
                        nv, t8[:, sl], 0.0, u, A.max, A.subtract
                    )
                    nvs[j] = nv

                for j in range(NCC):
                    sl = slice(j * CW, (j + 1) * CW)
                    xsl = xs[j // SUB][:, (j % SUB) * CW : (j % SUB + 1) * CW]
                    if t == T - 1 and j == NCC - 1:
                        u = pool.tile([128, CW], f32, tag="u", bufs=ubufs)
                        nq = 4
                        q = CW // nq
                        for k in range(nq):
                            usub = slice(k * q, (k + 1) * q)
                            ysub = slice(j * CW + k * q, j * CW + (k + 1) * q)
                            nc.vector.scalar_tensor_tensor(
                                u[:, usub], nvs[j][:, usub], -0.5,
                                xsl[:, usub], A.mult, A.add,
                            )
                            nc.scalar.activation(
                                t8[:, ysub], u[:, usub], AF.Sign, bias=bias
                            )
                            nc.sync.dma_start(out=y[t][:, ysub], in_=t8[:, ysub])
                        continue
                    if t == 0:
                        u = xsl
                    else:
                        u = pool.tile([128, CW], f32, tag="u", bufs=ubufs)
                        nc.vector.scalar_tensor_tensor(
                            u, nvs[j], -0.5, xsl, A.mult, A.add
                        )
                    if t == 0 and j < 2:
                        nc.vector.tensor_scalar(t8[:, sl], u, 1.0, None, A.is_ge)
                    else:
                        nc.scalar.activation(t8[:, sl], u, AF.Sign, bias=bias)
                    if t < T - 1:
                        if pending is not None:
                            emit_reset(*pending)
                        pending = (j, u)
                    else:
                        nc.sync.dma_start(out=y[t][:, sl], in_=t8[:, sl])
                if pending is not None:
                    emit_reset(*pending)

                if t < T - 1:
                    store_pending = (t, t8)
            if store_pending is not None:
                pt, pt8 = store_pending
                nc.sync.dma_start(out=y[pt], in_=pt8)

    nc.compile()
    return nc


def _get_nc():
    if "nc" not in _CACHE:
        scheme = os.environ.get("LIF_SCHEME", "psum")
        _CACHE["scheme"] = scheme
        _CACHE["nc"] = _build_sign() if scheme == "sign" else _build_psum()
    return _CACHE["nc"]


def kernel(x: np.ndarray) -> np.ndarray:
    global LAST_EXEC_NS, LAST_TRACE
    from concourse.bass_utils import run_bass_kernel_spmd

    x = np.ascontiguousarray(np.asarray(x), dtype=np.float32)
    assert x.shape == (T * B, C, 32, 32), x.shape
    xv = x.reshape(T, B, C, HW)

    nc = _get_nc()
    scheme = _CACHE.get("scheme", "psum")

    wI = None
    if scheme != "sign":
        import ml_dtypes

        wI = (np.eye(128, dtype=np.float32) * -0.5).astype(ml_dtypes.bfloat16)

    in_maps = []
    for m in range(NCORES):
        shard = np.ascontiguousarray(xv[:, m * BLOC : (m + 1) * BLOC]).reshape(
            T, 128, F
        )
        im = {"x": shard}
        if wI is not None:
            im["w"] = wI
        in_maps.append(im)

    trace = os.environ.get("LIF_TRACE") == "1"
    res = run_bass_kernel_spmd(nc, in_maps, core_ids=list(range(NCORES)), trace=trace)
    LAST_EXEC_NS = res.exec_time_ns
    if res.instructions_and_trace is not None:
        LAST_TRACE = res.instructions_and_trace[1]

    out = np.empty((T, B, C, HW), dtype=np.float32)
    for m in range(NCORES):
        raw = np.asarray(res.results[m]["y"])
        if scheme == "sign":
            sp = raw.view(np.int8) == 1
        else:
            # fp8e4 bytes: 1.0 = 0x38 (spike); 0x00 / 0xB8 (-1.0) = no spike
            sp = raw.view(np.uint8) == 0x38
        out[:, m * BLOC : (m + 1) * BLOC] = sp.astype(np.float32).reshape(
            T, BLOC, C, HW
        )
    return out.reshape(T * B, C, 32, 32)
